# revision 5
# baseline (speedup 1.0000x reference)
"""MultiInnerProductDecoder on 8 trn2 NeuronCores.

For each edge type t (8 of them) and edge e:
    out[t, e] = sigmoid( sum_d z[src[t,e], d] * z[dst[t,e], d] * weight[t, d] )

Sharding: one edge type per core. Each core gathers 2 x 100k z-rows with
InstDMAGatherAnt (SWDGE firmware gather). The kernel is descriptor-
GENERATION bound on the GpSimd Q7 cores (~8ns/row per queue pair,
~2.6ns/row aggregate over the 4 SWDGE queues), so:
  - z is cast to bf16 on host (256B rows) and the per-etype weight is
    folded into the src-side copy (zw = z * w[t]), halving SDMA packet
    bytes and removing one DVE pass; the dst side gathers plain bf16 z.
  - gathers alternate across all 4 SWDGE queues so all four Q7 core
    pairs generate descriptors concurrently.
  - 10 work buffers per tile tag keep the gather pipeline free of
    tile-reuse stalls (generation runs at the 2.09 ns/desc 4-queue
    aggregate limit, gap-free in the trace).
  - 2048-edge segments (129 ring slots of 1024) keep the decode-side
    `await_space` from blocking the Pool sequencer; bigger segments
    serialize all queues, smaller ones pay per-instruction overhead.
  - Indices preload in a head (first 4 segments) + rest split so the
    first gathers start early; big segments run first so the kernel
    tail is a small segment's drain.

dma_gather takes int16 indices, so node ids are split into four
32768-row chunks. The host sorts each core's edges into 16
(src_chunk, dst_chunk) buckets; within a bucket both endpoints are
addressed as `id & 0x7fff` against compile-time chunk bases. Buckets
are padded (with node-0 dummy edges) to the max bucket size across the
8 cores so a single SPMD program serves all cores. Bucket data is
processed in segments of up to SEG_E edges; gather position i lands at
out[i % 128, i // 128, :]. The host inverse-permutes the result.
"""

import ml_dtypes
import numpy as np

import concourse.mybir as mybir
from concourse import bacc
from concourse.bass_utils import run_bass_kernel_spmd
from concourse.tile import TileContext

N_NODES = 100000
NUM_ET = 8
D = 128
N_EDGES = 100000

CHUNK = 32768               # rows per index chunk (int16 reach)
N_CHUNKS = 4
SEG_E = 2048                # edges per segment
SEG_C = SEG_E // 128        # free-dim slots per partition at full segment
NQ = 4                      # SWDGE queues
SCRATCH = 16384
WORK_BUFS = 10
HEAD_SEGS = 8               # segments whose idxs load via the scalar HWDGE queue

F32 = mybir.dt.float32
BF16 = mybir.dt.bfloat16
I16 = mybir.dt.int16
NP_BF16 = ml_dtypes.bfloat16


def build_program(seg_specs):
    """seg_specs: tuple of (src_chunk, dst_chunk, num_idxs) per segment,
    num_idxs a multiple of 128, <= SEG_E. Same for all cores."""
    n_seg = len(seg_specs)
    W = SEG_E // 16
    nc = bacc.Bacc(num_swdge_queues=NQ, dynamic_dma_scratch_size=SCRATCH)

    zw = nc.declare_dram_parameter("zw", [N_NODES, D], BF16, isOutput=False)
    z = nc.declare_dram_parameter("z", [N_NODES, D], BF16, isOutput=False)
    idx_src = nc.declare_dram_parameter("idx_src", [128, n_seg * W], I16, isOutput=False)
    idx_dst = nc.declare_dram_parameter("idx_dst", [128, n_seg * W], I16, isOutput=False)
    out = nc.declare_dram_parameter("out", [n_seg * 128, SEG_C], F32, isOutput=True)

    with TileContext(nc) as tc:
        with (
            tc.tile_pool(name="const", bufs=1) as const_pool,
            tc.tile_pool(name="work", bufs=WORK_BUFS) as work,
            tc.tile_pool(name="small", bufs=4) as small,
        ):
            # Preload the wrapped indices. All transfers queued on one HWDGE
            # queue are served round-robin at descriptor granularity, so the
            # whole 4.2MB completes as one blob (~34us) no matter the issue
            # order -- a head load sharing the sync queue with the bulk
            # makes the FIRST gather wait ~33us for its 32KB. Fix: the head
            # (first HEAD_SEGS segments) goes on the SCALAR engine's
            # separate HWDGE queue, bypassing the bulk; it lands ~10us and
            # covers queue demand until the sync-queue bulk blob drains.
            h = min(HEAD_SEGS, n_seg)
            si_head = const_pool.tile([128, h * W], I16)
            di_head = const_pool.tile([128, h * W], I16)
            nc.scalar.dma_start(out=si_head[:], in_=idx_src[:, : h * W])
            nc.scalar.dma_start(out=di_head[:], in_=idx_dst[:, : h * W])
            si_rest = di_rest = None
            if n_seg > h:
                si_rest = const_pool.tile([128, (n_seg - h) * W], I16)
                di_rest = const_pool.tile([128, (n_seg - h) * W], I16)
                nc.sync.dma_start(out=si_rest[:], in_=idx_src[:, h * W :])
                nc.sync.dma_start(out=di_rest[:], in_=idx_dst[:, h * W :])

            def idx_view(all_head, all_rest, s):
                if s < h:
                    return all_head[:].rearrange("p (s w) -> p s w", s=h)[:, s]
                return all_rest[:].rearrange("p (s w) -> p s w", s=n_seg - h)[:, s - h]

            for s, (cs, cd, ni) in enumerate(seg_specs):
                c = ni // 128
                rows = slice(s * 128, (s + 1) * 128)

                zs = work.tile([128, c * D], BF16, tag="zs")
                zd = work.tile([128, c * D], BF16, tag="zd")
                src_base = cs * CHUNK
                dst_base = cd * CHUNK
                nc.gpsimd.dma_gather(
                    out_ap=zs[:].rearrange("p (c d) -> p c d", d=D),
                    in_ap=zw[src_base : min(src_base + CHUNK, N_NODES), :],
                    idxs_ap=idx_view(si_head, si_rest, s)[:, : ni // 16],
                    num_idxs=ni,
                    num_idxs_reg=ni,
                    elem_size=D,
                    single_packet=False,
                    queue_num=(2 * s) % NQ,
                )
                nc.gpsimd.dma_gather(
                    out_ap=zd[:].rearrange("p (c d) -> p c d", d=D),
                    in_ap=z[dst_base : min(dst_base + CHUNK, N_NODES), :],
                    idxs_ap=idx_view(di_head, di_rest, s)[:, : ni // 16],
                    num_idxs=ni,
                    num_idxs_reg=ni,
                    elem_size=D,
                    single_packet=False,
                    queue_num=(2 * s + 1) % NQ,
                )

                # prod <- zs * zd (bf16; NOT in-place: the 2-port DVE mode
                # races when out aliases an input); vals <- sum_d; sigmoid
                prod = work.tile([128, c * D], BF16, tag="pr")
                nc.vector.tensor_tensor(
                    out=prod[:], in0=zs[:], in1=zd[:], op=mybir.AluOpType.mult
                )
                vals = small.tile([128, c], F32, tag="vals")
                nc.vector.tensor_reduce(
                    out=vals[:],
                    in_=prod[:].rearrange("p (c d) -> p c d", d=D),
                    axis=mybir.AxisListType.X,
                    op=mybir.AluOpType.add,
                )
                res = small.tile([128, c], F32, tag="res")
                nc.scalar.activation(
                    out=res[:], in_=vals[:], func=mybir.ActivationFunctionType.Sigmoid
                )
                nc.sync.dma_start(out=out[rows, :c], in_=res[:])

    # Tile round-robins the 8 DMASW sem lanes over Pool-DMA instructions in
    # scheduled order, and the SWDGE firmware requires each sem lane to stay
    # on one queue. Derive queue_num from the assigned lane so the pairing
    # is consistent and the 4 queues stay evenly loaded.
    for f in nc.m.functions:
        for b in f.blocks:
            for ins in b.instructions:
                if isinstance(ins, mybir.InstDMAGatherAnt) and ins.sync_info:
                    for u in ins.sync_info.on_update or []:
                        name = getattr(u, "ant_name", "") or ""
                        if name.startswith("DMASW"):
                            ins.queue_num = int(name[5:].split("_")[0]) % NQ
                            break

    nc.finalize()
    return nc


_PROGRAMS = {}


def _get_program(seg_specs):
    key = tuple(seg_specs)
    if key not in _PROGRAMS:
        _PROGRAMS[key] = build_program(seg_specs)
    return _PROGRAMS[key]


def _wrap16(flat):
    """[n] int16 gather-position order -> [128, n/16] wrapped+replicated."""
    n = flat.shape[0]
    w = flat.reshape(n // 16, 16).T  # [16, n/16]; position i at [i%16, i//16]
    return np.tile(w, (8, 1))


def prepare(z, weight, edge_src, edge_dst):
    """Host-side bucketing. Returns (in_maps, seg_specs, recover)."""
    z = np.ascontiguousarray(np.asarray(z, dtype=np.float32))
    weight = np.asarray(weight, dtype=np.float32)
    src = np.asarray(edge_src, dtype=np.int64)
    dst = np.asarray(edge_dst, dtype=np.int64)
    n_edges = src.shape[1]

    z_bf = np.ascontiguousarray(z.astype(NP_BF16))

    key = (src >> 15) * N_CHUNKS + (dst >> 15)          # [T, E] bucket 0..15
    orders = [np.argsort(key[t], kind="stable") for t in range(NUM_ET)]
    counts = np.stack(
        [np.bincount(key[t], minlength=16) for t in range(NUM_ET)]
    )  # [T, 16]
    gsize = counts.max(axis=0)                           # padded bucket sizes
    gsize = ((gsize + 127) // 128) * 128
    gbase = np.concatenate([[0], np.cumsum(gsize)])      # [17]
    total = int(gbase[-1])

    # segment layout (same for all cores); big segments first so the
    # kernel tail is a small segment's drain+compute
    seg_specs = []
    seg_group_off = []                                   # (group, offset) per seg
    for g in range(16):
        sz = int(gsize[g])
        off = 0
        while off < sz:
            ni = min(SEG_E, sz - off)
            seg_specs.append((g // N_CHUNKS, g % N_CHUNKS, ni))
            seg_group_off.append((g, off))
            off += ni
    order_s = sorted(range(len(seg_specs)), key=lambda s: -seg_specs[s][2])
    seg_specs = [seg_specs[s] for s in order_s]
    seg_group_off = [seg_group_off[s] for s in order_s]
    n_seg = len(seg_specs)
    W = SEG_E // 16

    in_maps = []
    padpos_all = []
    for t in range(NUM_ET):
        order = orders[t]
        cnt = counts[t]
        within = np.concatenate([np.arange(cnt[g]) for g in range(16)])
        bases = np.repeat(gbase[:16], cnt)
        padpos_sorted = bases + within
        padpos = np.empty(n_edges, dtype=np.int64)
        padpos[order] = padpos_sorted
        padpos_all.append(padpos)

        # pad positions get idx -1: the gather firmware trims trailing
        # negative indices, skipping descriptor generation for full
        # 128-index pad chunks (per-core counts differ from the padded max)
        src_loc = np.zeros(total, dtype=np.int16)
        dst_loc = np.zeros(total, dtype=np.int16)
        src_loc[padpos] = (src[t] & 0x7FFF).astype(np.int16)
        dst_loc[padpos] = (dst[t] & 0x7FFF).astype(np.int16)

        idx_src_np = np.zeros((128, n_seg * W), dtype=np.int16)
        idx_dst_np = np.zeros((128, n_seg * W), dtype=np.int16)
        for s, ((g, off), (_, _, ni)) in enumerate(zip(seg_group_off, seg_specs)):
            lo = int(gbase[g]) + off
            idx_src_np[:, s * W : s * W + ni // 16] = _wrap16(src_loc[lo : lo + ni])
            idx_dst_np[:, s * W : s * W + ni // 16] = _wrap16(dst_loc[lo : lo + ni])

        in_maps.append(
            {
                "zw": np.ascontiguousarray((z * weight[t]).astype(NP_BF16)),
                "z": z_bf,
                "idx_src": idx_src_np,
                "idx_dst": idx_dst_np,
            }
        )

    recover = (seg_specs, seg_group_off, gbase, padpos_all, n_edges)
    return in_maps, tuple(seg_specs), recover


def recover_output(results, recover):
    seg_specs, seg_group_off, gbase, padpos_all, n_edges = recover
    total = int(gbase[-1])
    outs = []
    for t in range(NUM_ET):
        out_dram = results[t]["out"]                     # [n_seg*128, SEG_C]
        vals_padded = np.empty(total, dtype=np.float32)
        for s, ((g, off), (_, _, ni)) in enumerate(zip(seg_group_off, seg_specs)):
            seg = out_dram[s * 128 : (s + 1) * 128, : ni // 128]
            lo = int(gbase[g]) + off
            vals_padded[lo : lo + ni] = seg.T.ravel()    # position i at [i%128,i//128]
        outs.append(vals_padded[padpos_all[t]])
    return np.stack(outs).astype(np.float32)


def kernel(z, weight, edge_src, edge_dst):
    in_maps, seg_specs, recover = prepare(z, weight, edge_src, edge_dst)
    nc = _get_program(seg_specs)
    res = run_bass_kernel_spmd(nc, in_maps, core_ids=list(range(NUM_ET)))
    return recover_output(res.results, recover)



# revision 10
# speedup vs baseline: 1.0480x; 1.0480x over previous
"""MultiInnerProductDecoder on 8 trn2 NeuronCores.

For each edge type t (8 of them) and edge e:
    out[t, e] = sigmoid( sum_d z[src[t,e], d] * z[dst[t,e], d] * weight[t, d] )

Sharding: one edge type per core. Each core gathers 2 x 100k z-rows with
InstDMAGatherAnt (SWDGE firmware gather). The kernel is descriptor-
GENERATION bound on the GpSimd Q7 cores (~8ns/row per queue pair,
~2.6ns/row aggregate over the 4 SWDGE queues), so:
  - z is cast to bf16 on host (256B rows) and the per-etype weight is
    folded into the src-side copy (zw = z * w[t]), halving SDMA packet
    bytes and removing one DVE pass; the dst side gathers plain bf16 z.
  - gathers alternate across all 4 SWDGE queues so all four Q7 core
    pairs generate descriptors concurrently.
  - 10 work buffers per tile tag keep the gather pipeline free of
    tile-reuse stalls (generation runs at the 2.09 ns/desc 4-queue
    aggregate limit, gap-free in the trace).
  - 2048-edge segments (129 ring slots of 1024) keep the decode-side
    `await_space` from blocking the Pool sequencer; bigger segments
    serialize all queues, smaller ones pay per-instruction overhead.
  - Indices preload in a head (first 4 segments) + rest split so the
    first gathers start early; big segments run first so the kernel
    tail is a small segment's drain.

dma_gather takes int16 indices, so node ids are split into four
32768-row chunks. The host sorts each core's edges into 16
(src_chunk, dst_chunk) buckets; within a bucket both endpoints are
addressed as `id & 0x7fff` against compile-time chunk bases. Buckets
are padded (with node-0 dummy edges) to the max bucket size across the
8 cores so a single SPMD program serves all cores. Bucket data is
processed in segments of up to SEG_E edges; gather position i lands at
out[i % 128, i // 128, :]. The host inverse-permutes the result.
"""

import ml_dtypes
import numpy as np

import concourse.mybir as mybir
from concourse import bacc
from concourse.bass_utils import run_bass_kernel_spmd
from concourse.tile import TileContext

N_NODES = 100000
NUM_ET = 8
D = 128
N_EDGES = 100000

CHUNK = 32768               # rows per index chunk (int16 reach)
N_CHUNKS = 4
SEG_E = 4096                # edges per segment
SEG_C = SEG_E // 128        # free-dim slots per partition at full segment
NQ = 4                      # SWDGE queues
SCRATCH = 32768
WORK_BUFS = 5
HEAD_SEGS = 4               # segments whose idxs load in a separate first DMA

F32 = mybir.dt.float32
BF16 = mybir.dt.bfloat16
I16 = mybir.dt.int16
NP_BF16 = ml_dtypes.bfloat16


def build_program(seg_specs):
    """seg_specs: tuple of (src_chunk, dst_chunk, num_idxs) per segment,
    num_idxs a multiple of 128, <= SEG_E. Same for all cores."""
    n_seg = len(seg_specs)
    W = SEG_E // 16
    nc = bacc.Bacc(num_swdge_queues=NQ, dynamic_dma_scratch_size=SCRATCH)

    zw = nc.declare_dram_parameter("zw", [N_NODES, D], BF16, isOutput=False)
    z = nc.declare_dram_parameter("z", [N_NODES, D], BF16, isOutput=False)
    idx_src = nc.declare_dram_parameter("idx_src", [128, n_seg * W], I16, isOutput=False)
    idx_dst = nc.declare_dram_parameter("idx_dst", [128, n_seg * W], I16, isOutput=False)
    out = nc.declare_dram_parameter("out", [n_seg * 128, SEG_C], F32, isOutput=True)

    with TileContext(nc) as tc:
        with (
            tc.tile_pool(name="const", bufs=1) as const_pool,
            tc.tile_pool(name="work", bufs=WORK_BUFS) as work,
            tc.tile_pool(name="small", bufs=4) as small,
        ):
            # Preload the wrapped indices: a small head transfer first so
            # the first gathers start without waiting for the full 3MB.
            # (All transfers on the HW-dynamic path drain as one ~34us blob
            # -- packet-level round-robin across queues AND engines -- so
            # fancier head arrangements, incl. a separate scalar-engine
            # queue, were measured to not help.)
            h = min(HEAD_SEGS, n_seg)
            si_head = const_pool.tile([128, h * W], I16)
            di_head = const_pool.tile([128, h * W], I16)
            nc.sync.dma_start(out=si_head[:], in_=idx_src[:, : h * W])
            nc.sync.dma_start(out=di_head[:], in_=idx_dst[:, : h * W])
            si_rest = di_rest = None
            if n_seg > h:
                si_rest = const_pool.tile([128, (n_seg - h) * W], I16)
                di_rest = const_pool.tile([128, (n_seg - h) * W], I16)
                nc.sync.dma_start(out=si_rest[:], in_=idx_src[:, h * W :])
                nc.sync.dma_start(out=di_rest[:], in_=idx_dst[:, h * W :])

            def idx_view(all_head, all_rest, s):
                if s < h:
                    return all_head[:].rearrange("p (s w) -> p s w", s=h)[:, s]
                return all_rest[:].rearrange("p (s w) -> p s w", s=n_seg - h)[:, s - h]

            for s, (cs, cd, ni) in enumerate(seg_specs):
                c = ni // 128
                rows = slice(s * 128, (s + 1) * 128)

                zs = work.tile([128, c * D], BF16, tag="zs")
                zd = work.tile([128, c * D], BF16, tag="zd")
                src_base = cs * CHUNK
                dst_base = cd * CHUNK
                nc.gpsimd.dma_gather(
                    out_ap=zs[:].rearrange("p (c d) -> p c d", d=D),
                    in_ap=zw[src_base : min(src_base + CHUNK, N_NODES), :],
                    idxs_ap=idx_view(si_head, si_rest, s)[:, : ni // 16],
                    num_idxs=ni,
                    num_idxs_reg=ni,
                    elem_size=D,
                    single_packet=False,
                    queue_num=(2 * s) % NQ,
                )
                nc.gpsimd.dma_gather(
                    out_ap=zd[:].rearrange("p (c d) -> p c d", d=D),
                    in_ap=z[dst_base : min(dst_base + CHUNK, N_NODES), :],
                    idxs_ap=idx_view(di_head, di_rest, s)[:, : ni // 16],
                    num_idxs=ni,
                    num_idxs_reg=ni,
                    elem_size=D,
                    single_packet=False,
                    queue_num=(2 * s + 1) % NQ,
                )

                # prod <- zs * zd (bf16; NOT in-place: the 2-port DVE mode
                # races when out aliases an input); vals <- sum_d; sigmoid
                prod = work.tile([128, c * D], BF16, tag="pr")
                nc.vector.tensor_tensor(
                    out=prod[:], in0=zs[:], in1=zd[:], op=mybir.AluOpType.mult
                )
                vals = small.tile([128, c], F32, tag="vals")
                nc.vector.tensor_reduce(
                    out=vals[:],
                    in_=prod[:].rearrange("p (c d) -> p c d", d=D),
                    axis=mybir.AxisListType.X,
                    op=mybir.AluOpType.add,
                )
                res = small.tile([128, c], F32, tag="res")
                nc.scalar.activation(
                    out=res[:], in_=vals[:], func=mybir.ActivationFunctionType.Sigmoid
                )
                nc.sync.dma_start(out=out[rows, :c], in_=res[:])

    # Tile round-robins the 8 DMASW sem lanes over Pool-DMA instructions in
    # scheduled order, and the SWDGE firmware requires each sem lane to stay
    # on one queue. Derive queue_num from the assigned lane so the pairing
    # is consistent and the 4 queues stay evenly loaded.
    for f in nc.m.functions:
        for b in f.blocks:
            for ins in b.instructions:
                if isinstance(ins, mybir.InstDMAGatherAnt) and ins.sync_info:
                    for u in ins.sync_info.on_update or []:
                        name = getattr(u, "ant_name", "") or ""
                        if name.startswith("DMASW"):
                            ins.queue_num = int(name[5:].split("_")[0]) % NQ
                            break

    nc.finalize()
    return nc


_PROGRAMS = {}


def _get_program(seg_specs):
    key = tuple(seg_specs)
    if key not in _PROGRAMS:
        _PROGRAMS[key] = build_program(seg_specs)
    return _PROGRAMS[key]


def _wrap16(flat):
    """[n] int16 gather-position order -> [128, n/16] wrapped+replicated."""
    n = flat.shape[0]
    w = flat.reshape(n // 16, 16).T  # [16, n/16]; position i at [i%16, i//16]
    return np.tile(w, (8, 1))


def prepare(z, weight, edge_src, edge_dst):
    """Host-side bucketing. Returns (in_maps, seg_specs, recover)."""
    z = np.ascontiguousarray(np.asarray(z, dtype=np.float32))
    weight = np.asarray(weight, dtype=np.float32)
    src = np.asarray(edge_src, dtype=np.int64)
    dst = np.asarray(edge_dst, dtype=np.int64)
    n_edges = src.shape[1]

    z_bf = np.ascontiguousarray(z.astype(NP_BF16))

    key = (src >> 15) * N_CHUNKS + (dst >> 15)          # [T, E] bucket 0..15
    orders = [np.argsort(key[t], kind="stable") for t in range(NUM_ET)]
    counts = np.stack(
        [np.bincount(key[t], minlength=16) for t in range(NUM_ET)]
    )  # [T, 16]
    gsize = counts.max(axis=0)                           # padded bucket sizes
    gsize = ((gsize + 127) // 128) * 128
    gbase = np.concatenate([[0], np.cumsum(gsize)])      # [17]
    total = int(gbase[-1])

    # segment layout (same for all cores); big segments first so the
    # kernel tail is a small segment's drain+compute
    seg_specs = []
    seg_group_off = []                                   # (group, offset) per seg
    for g in range(16):
        sz = int(gsize[g])
        off = 0
        while off < sz:
            ni = min(SEG_E, sz - off)
            seg_specs.append((g // N_CHUNKS, g % N_CHUNKS, ni))
            seg_group_off.append((g, off))
            off += ni
    order_s = sorted(range(len(seg_specs)), key=lambda s: -seg_specs[s][2])
    seg_specs = [seg_specs[s] for s in order_s]
    seg_group_off = [seg_group_off[s] for s in order_s]
    n_seg = len(seg_specs)
    W = SEG_E // 16

    in_maps = []
    padpos_all = []
    for t in range(NUM_ET):
        order = orders[t]
        cnt = counts[t]
        within = np.concatenate([np.arange(cnt[g]) for g in range(16)])
        bases = np.repeat(gbase[:16], cnt)
        padpos_sorted = bases + within
        padpos = np.empty(n_edges, dtype=np.int64)
        padpos[order] = padpos_sorted
        padpos_all.append(padpos)

        # pad positions get idx -1: the gather firmware trims trailing
        # negative indices, skipping descriptor generation for full
        # 128-index pad chunks (per-core counts differ from the padded max)
        src_loc = np.zeros(total, dtype=np.int16)
        dst_loc = np.zeros(total, dtype=np.int16)
        src_loc[padpos] = (src[t] & 0x7FFF).astype(np.int16)
        dst_loc[padpos] = (dst[t] & 0x7FFF).astype(np.int16)

        idx_src_np = np.zeros((128, n_seg * W), dtype=np.int16)
        idx_dst_np = np.zeros((128, n_seg * W), dtype=np.int16)
        for s, ((g, off), (_, _, ni)) in enumerate(zip(seg_group_off, seg_specs)):
            lo = int(gbase[g]) + off
            idx_src_np[:, s * W : s * W + ni // 16] = _wrap16(src_loc[lo : lo + ni])
            idx_dst_np[:, s * W : s * W + ni // 16] = _wrap16(dst_loc[lo : lo + ni])

        in_maps.append(
            {
                "zw": np.ascontiguousarray((z * weight[t]).astype(NP_BF16)),
                "z": z_bf,
                "idx_src": idx_src_np,
                "idx_dst": idx_dst_np,
            }
        )

    recover = (seg_specs, seg_group_off, gbase, padpos_all, n_edges)
    return in_maps, tuple(seg_specs), recover


def recover_output(results, recover):
    seg_specs, seg_group_off, gbase, padpos_all, n_edges = recover
    total = int(gbase[-1])
    outs = []
    for t in range(NUM_ET):
        out_dram = results[t]["out"]                     # [n_seg*128, SEG_C]
        vals_padded = np.empty(total, dtype=np.float32)
        for s, ((g, off), (_, _, ni)) in enumerate(zip(seg_group_off, seg_specs)):
            seg = out_dram[s * 128 : (s + 1) * 128, : ni // 128]
            lo = int(gbase[g]) + off
            vals_padded[lo : lo + ni] = seg.T.ravel()    # position i at [i%128,i//128]
        outs.append(vals_padded[padpos_all[t]])
    return np.stack(outs).astype(np.float32)


def kernel(z, weight, edge_src, edge_dst):
    in_maps, seg_specs, recover = prepare(z, weight, edge_src, edge_dst)
    nc = _get_program(seg_specs)
    res = run_bass_kernel_spmd(nc, in_maps, core_ids=list(range(NUM_ET)))
    return recover_output(res.results, recover)

